# revision 1
# baseline (speedup 1.0000x reference)
"""CTC loss (keras ctc_batch_cost semantics) on 8 Trainium2 NeuronCores.

Strategy (pure data parallelism, batch sharded 8 ways):
  - Device kernel runs the CTC forward DP in probability space with periodic
    max-rescaling (scale logs accumulated, summed at the end).
  - The per-batch gather q[b, t, ext[b, s]] is done on GPSIMD via ap_gather
    in a t-on-partitions layout (indices depend only on b, so they are
    shared across all 128 t-partitions — exactly the ap_gather contract),
    then PE transposes + ScalarE PSUM->SBUF copies regroup into the
    b-on-partitions layout the DP needs.
  - DP inner loop: 4 tensor_tensor ops per time step on [128, 4x33] tiles
    (batch 512 per core = 128 partitions x 4 groups), rescale every R steps.

Self-contained: hardcodes shapes from the problem spec.
"""

import numpy as np

# Problem dims (hardcoded per spec nn_CTCLayer_4518305595673)
B, T, C, L = 4096, 128, 96, 16
NCORES = 8
BC = B // NCORES            # 512 batches per core
S = 2 * L + 1               # 33 extended label positions
G4 = BC // 128              # 4 partition groups
BLANK = C - 1               # 95
EPS = 1e-7
R = 4                       # rescale every R time steps
GB = 64                     # batches per ap_gather call (HW-validated:
                            # fewer, larger ap_gather calls are much faster;
                            # 64 -> ~105us/iter vs 32 -> ~401us, 16 -> ~2.2ms)

_CACHE = {}


def _wc_pad(gb, s_len):
    """Wrapped idx columns per gather group, padded to 4-byte alignment."""
    wc = gb * s_len // 16
    return wc + (wc % 2)


def _build_program(bc=BC, t_len=T, c_dim=C, l_len=L, r_period=R, gb=GB,
                   v_gpsimd=False, repeat=1):
    """Build + compile the per-core Bass program."""
    import concourse.bacc as bacc
    import concourse.tile as tile
    from concourse import masks, mybir
    from contextlib import ExitStack

    s_len = 2 * l_len + 1
    sg = s_len + 2
    g4 = bc // 128
    nbg = bc // gb              # gather groups per core
    jpt = 128 // gb             # gather groups per 128-batch tile
    wc = gb * s_len // 16       # wrapped idx columns actually read
    wcp = _wc_pad(gb, s_len)    # stored (padded) columns
    ts = t_len * s_len
    resc_ts = sorted(set([t for t in range(1, t_len) if t % r_period == 0]
                         + [t_len - 1]))
    nsl = len(resc_ts)

    f32 = mybir.dt.float32
    i16 = mybir.dt.int16
    Alu = mybir.AluOpType
    Act = mybir.ActivationFunctionType
    Ax = mybir.AxisListType

    nc = bacc.Bacc("TRN2", target_bir_lowering=False, debug=False,
                   num_devices=NCORES)
    yp = nc.dram_tensor("yp", [bc, t_len, c_dim], f32, kind="ExternalInput")
    gidx = nc.dram_tensor("gidx", [128, nbg * wcp], i16, kind="ExternalInput")
    msk = nc.dram_tensor("mask", [128, g4 * s_len], f32, kind="ExternalInput")
    loss = nc.dram_tensor("loss", [bc, 1], f32, kind="ExternalOutput")

    with tile.TileContext(nc) as tc, ExitStack() as ctx:
        const_pool = ctx.enter_context(tc.tile_pool(name="const", bufs=1))
        load_pool = ctx.enter_context(tc.tile_pool(name="load", bufs=3))
        g_pool = ctx.enter_context(tc.tile_pool(name="gath", bufs=2))
        psum_pool = ctx.enter_context(
            tc.tile_pool(name="ps", bufs=4, space="PSUM"))
        big_pool = ctx.enter_context(tc.tile_pool(name="big", bufs=1))
        dp_pool = ctx.enter_context(tc.tile_pool(name="dp", bufs=1))

        ident = const_pool.tile([128, 128], f32)
        masks.make_identity(nc, ident[:])
        gidx_sb = const_pool.tile([128, nbg * wcp], i16)
        nc.sync.dma_start(gidx_sb[:], gidx.ap())
        mask_sb = const_pool.tile([128, g4 * s_len], f32)
        nc.sync.dma_start(mask_sb[:], msk.ap())
        mv = mask_sb[:].rearrange("p (g s) -> p g s", g=g4)

        def body():
            qe = big_pool.tile([128, g4 * ts], f32, tag="qe")
            qev = qe[:].rearrange("p (g t s) -> p g t s", g=g4, t=t_len)

            # ---- gather + regroup phase ----
            for bt in range(g4):
                g_tile = g_pool.tile([128, 128 * s_len], f32, tag="gt")
                for jj in range(jpt):
                    j = bt * jpt + jj
                    l_tile = load_pool.tile([128, gb * c_dim], f32, tag="ld")
                    src = (yp.ap()[gb * j:gb * (j + 1)]
                           .rearrange("b t c -> t b c"))
                    dst = l_tile[:].rearrange("p (b c) -> p b c", b=gb)
                    nc.sync.dma_start(dst, src)
                    nc.gpsimd.ap_gather(
                        out_ap=g_tile[:, jj * gb * s_len:(jj + 1) * gb * s_len],
                        in_ap=l_tile[:],
                        idxs_ap=gidx_sb[:, j * wcp:j * wcp + wc],
                        channels=128,
                        num_elems=gb * c_dim,
                        d=1,
                        num_idxs=gb * s_len,
                    )
                gv = g_tile[:].rearrange("p (j i s) -> p j i s", j=jpt, i=gb)
                for s in range(s_len):
                    pt = psum_pool.tile([128, 128], f32, tag="pt")
                    nc.tensor.transpose(pt[:], gv[:, :, :, s], ident[:])
                    nc.scalar.activation(qev[:, bt, :, s], pt[:], Act.Copy,
                                         bias=EPS)

            # ---- DP phase ----
            alpha_a = dp_pool.tile([128, g4 * sg], f32, tag="alpha_a")
            alpha_b = dp_pool.tile([128, g4 * sg], f32, tag="alpha_b")
            a_tiles = [alpha_a, alpha_b]
            for a in a_tiles:
                nc.vector.memset(a[:], 0.0)
            av = [a[:].rearrange("p (g s) -> p g s", g=g4) for a in a_tiles]

            u_t = dp_pool.tile([128, g4 * s_len], f32, tag="u_t")
            v_t = dp_pool.tile([128, g4 * s_len], f32, tag="v_t")
            uv = u_t[:].rearrange("p (g s) -> p g s", g=g4)
            vv = v_t[:].rearrange("p (g s) -> p g s", g=g4)

            scl = dp_pool.tile([128, g4 * nsl], f32, tag="scl")
            sclv = scl[:].rearrange("p (g n) -> p g n", g=g4)
            rec = dp_pool.tile([128, g4], f32, tag="rec")

            # t = 0 init: alpha[s=0,1] = q'[0, s], rest 0
            nc.vector.tensor_copy(av[0][:, :, 2:4], qev[:, :, 0, 0:2])

            cur = 0
            for t in range(1, t_len):
                prev, nxt = av[cur], av[1 - cur]
                nc.vector.tensor_tensor(uv[:, :, :], prev[:, :, 2:2 + s_len],
                                        prev[:, :, 1:1 + s_len], op=Alu.add)
                veng = nc.gpsimd if v_gpsimd else nc.vector
                veng.tensor_tensor(vv[:, :, :], prev[:, :, 0:s_len],
                                   mv[:, :, :], op=Alu.mult)
                nc.vector.tensor_tensor(uv[:, :, :], uv[:, :, :], vv[:, :, :],
                                        op=Alu.add)
                nc.vector.tensor_tensor(nxt[:, :, 2:2 + s_len], uv[:, :, :],
                                        qev[:, :, t, :], op=Alu.mult)
                if t in resc_ts:
                    slot = resc_ts.index(t)
                    nc.vector.tensor_reduce(sclv[:, :, slot],
                                            nxt[:, :, 2:2 + s_len],
                                            axis=Ax.X, op=Alu.max)
                    nc.vector.reciprocal(rec[:], sclv[:, :, slot])
                    bb = rec[:].unsqueeze(2).broadcast_to((128, g4, s_len))
                    nc.vector.tensor_tensor(nxt[:, :, 2:2 + s_len],
                                            nxt[:, :, 2:2 + s_len], bb,
                                            op=Alu.mult)
                cur = 1 - cur

            # ---- epilogue ----
            lg = dp_pool.tile([128, g4 * nsl], f32, tag="lg")
            nc.scalar.activation(lg[:], scl[:], Act.Ln)
            lsum = dp_pool.tile([128, g4], f32, tag="lsum")
            nc.vector.tensor_reduce(lsum[:],
                                    lg[:].rearrange("p (g n) -> p g n", g=g4),
                                    axis=Ax.X, op=Alu.add)
            tail = dp_pool.tile([128, g4], f32, tag="tail")
            fin = av[cur]
            nc.vector.tensor_tensor(tail[:], fin[:, :, sg - 2],
                                    fin[:, :, sg - 1], op=Alu.add)
            ltail = dp_pool.tile([128, g4], f32, tag="ltail")
            nc.scalar.activation(ltail[:], tail[:], Act.Ln)
            tot = dp_pool.tile([128, g4], f32, tag="tot")
            nc.vector.tensor_tensor(tot[:], lsum[:], ltail[:], op=Alu.add)
            loss_sb = dp_pool.tile([128, g4], f32, tag="loss_sb")
            nc.vector.tensor_scalar_mul(loss_sb[:], tot[:], -1.0)
            nc.sync.dma_start(
                loss.ap().rearrange("(g p) one -> p (g one)", p=128),
                loss_sb[:])

        for _rep in range(repeat):
            body()

    nc.compile()
    return nc


def _host_prep(y_true, y_pred, bc=BC, gb=GB, s_len=S):
    """Shard + build index/mask tensors. Returns in_maps list."""
    y_true = np.asarray(y_true).astype(np.int64)
    y_pred = np.ascontiguousarray(np.asarray(y_pred), dtype=np.float32)
    ncores = y_pred.shape[0] // bc
    g4 = bc // 128
    nbg = bc // gb
    wc = gb * s_len // 16
    wcp = _wc_pad(gb, s_len)
    ext = np.full((y_true.shape[0], s_len), BLANK, dtype=np.int64)
    ext[:, 1::2] = y_true
    mask_full = np.zeros((ext.shape[0], s_len), dtype=np.float32)
    mask_full[:, 2:] = ((ext[:, 2:] != ext[:, :-2])
                        & (ext[:, 2:] != BLANK)).astype(np.float32)

    in_maps = []
    for cid in range(ncores):
        b0 = cid * bc
        yp_c = y_pred[b0:b0 + bc]
        ext_c = ext[b0:b0 + bc]
        # gather indices: per gb-batch group j, idxlist[(i, s)] = i*C + ext;
        # wrapped in 16 partitions: stored[p%16, w] = idxlist[w*16 + p%16].
        # Each group's slice is padded to an even column count so every
        # slice base is 4-byte aligned (ap_gather HW requirement).
        gidx_c = np.zeros((128, nbg * wcp), dtype=np.int16)
        for j in range(nbg):
            idxlist = (np.arange(gb, dtype=np.int16)[:, None] * C
                       + ext_c[gb * j:gb * (j + 1)].astype(np.int16))
            wrapped = idxlist.reshape(-1).reshape(wc, 16).T  # [p16, w]
            gidx_c[:, j * wcp:j * wcp + wc] = np.tile(wrapped, (8, 1))
        m = mask_full[b0:b0 + bc].reshape(g4, 128, s_len).transpose(1, 0, 2)
        mask_c = np.ascontiguousarray(m.reshape(128, g4 * s_len))
        in_maps.append({"yp": yp_c, "gidx": gidx_c, "mask": mask_c})
    return in_maps


def get_program(repeat=1):
    key = ("nc", repeat)
    if key not in _CACHE:
        _CACHE[key] = _build_program(repeat=repeat)
    return _CACHE[key]


def kernel(y_true, y_pred):
    from concourse import bass_utils
    nc = get_program()
    in_maps = _host_prep(y_true, y_pred)
    res = bass_utils.run_bass_kernel_spmd(nc, in_maps,
                                          core_ids=list(range(NCORES)))
    out = np.concatenate([res.results[c]["loss"] for c in range(NCORES)],
                         axis=0)
    return out.astype(np.float32)



# revision 4
# speedup vs baseline: 1.2813x; 1.2813x over previous
"""CTC loss (keras ctc_batch_cost semantics) on 8 Trainium2 NeuronCores.

Strategy (pure data parallelism, batch sharded 8 ways; 512 batches/core):
  - y_pred ships to the device as bf16 (halves HBM traffic; validated
    numerically: max rel err ~2e-4 vs the f32 reference, tolerance 2e-2).
  - Per 64-batch chunk: DMA bf16 [128t, 64b*96c] -> Act engine upcasts to
    f32 -> GPSIMD ap_gather picks the 33 extended-label classes per batch
    (t-on-partitions layout: indices depend only on b, shared across t).
  - Per 128-batch group: PE transposes regroup [t, b] -> [b, t] per s,
    4 s-slices packed per PSUM bank, one Act copy per bank writes the
    DP input qe[b, g, t, s] in bf16 (bias = EPS folded in).
  - DP: probability-space forward recurrence in bf16 (2x DVE mode),
    full-width [128, 4*33] ops; the allow2-mask multiply runs on Pool for
    a fraction of steps to balance engines; shared-scale rescale (one
    scale per partition row) every R=8 steps tracked via reciprocals.
  - Pools are >=2-buffered so iteration n+1's DMA/upcast/gather/transpose
    overlaps iteration n's DP across engines.

Self-contained: hardcodes shapes from the problem spec.
"""

import numpy as np

# Problem dims (hardcoded per spec nn_CTCLayer_4518305595673)
B, T, C, L = 4096, 128, 96, 16
NCORES = 8
BC = B // NCORES            # 512 batches per core
S = 2 * L + 1               # 33 extended label positions
G4 = BC // 128              # 4 partition groups
BLANK = C - 1               # 95
EPS = 1e-7
R = 8                       # rescale every R time steps (R=16 underflows)
GB = 64                     # batches per DMA/upcast/ap_gather chunk
POOL_MASK_K = 2             # of every 4 steps, this many run mask-mult on Pool

_CACHE = {}


def _build_program(bc=BC, t_len=T, c_dim=C, l_len=L, r_period=R, gb=GB,
                   pool_mask_k=POOL_MASK_K, repeat=1):
    """Build + compile the per-core Bass program."""
    import concourse.bacc as bacc
    import concourse.tile as tile
    from concourse import masks, mybir
    from contextlib import ExitStack

    s_len = 2 * l_len + 1
    sg = s_len + 2              # per-group alpha cols: 2 pad + s_len
    g4 = bc // 128
    nbg = bc // gb              # gather chunks per core (8)
    jpt = 128 // gb             # chunks per 128-batch group (2)
    wc = gb * s_len // 16       # wrapped idx columns (132, even)
    resc_ts = sorted(set([t for t in range(1, t_len) if t % r_period == 0]
                         + [t_len - 1]))
    nsl = len(resc_ts)
    pk4 = 4                     # transposed s-slices packed per PSUM copy

    f32 = mybir.dt.float32
    bf16 = mybir.dt.bfloat16
    i16 = mybir.dt.int16
    Alu = mybir.AluOpType
    Act = mybir.ActivationFunctionType
    Ax = mybir.AxisListType

    nc = bacc.Bacc("TRN2", target_bir_lowering=False, debug=False,
                   num_devices=NCORES)
    yp = nc.dram_tensor("yp", [bc, t_len, c_dim], bf16, kind="ExternalInput")
    gidx = nc.dram_tensor("gidx", [128, nbg * wc], i16, kind="ExternalInput")
    msk = nc.dram_tensor("mask", [128, g4 * s_len], bf16, kind="ExternalInput")
    loss = nc.dram_tensor("loss", [bc, 1], f32, kind="ExternalOutput")

    with tile.TileContext(nc) as tc, ExitStack() as ctx:
        const_pool = ctx.enter_context(tc.tile_pool(name="const", bufs=1))
        load_pool = ctx.enter_context(tc.tile_pool(name="load", bufs=2))
        up_pool = ctx.enter_context(tc.tile_pool(name="up", bufs=2))
        g_pool = ctx.enter_context(tc.tile_pool(name="gath", bufs=2))
        psum_pool = ctx.enter_context(
            tc.tile_pool(name="ps", bufs=4, space="PSUM"))
        qe_pool = ctx.enter_context(tc.tile_pool(name="qe", bufs=2))
        dp_pool = ctx.enter_context(tc.tile_pool(name="dp", bufs=2))

        ident = const_pool.tile([128, 128], f32)
        masks.make_identity(nc, ident[:])
        gidx_sb = const_pool.tile([128, nbg * wc], i16)
        nc.sync.dma_start(gidx_sb[:], gidx.ap())
        mask_sb = const_pool.tile([128, g4 * s_len], bf16)
        nc.sync.dma_start(mask_sb[:], msk.ap())
        mv = mask_sb[:].rearrange("p (g s) -> p g s", g=g4)

        def body():
            qe = qe_pool.tile([128, g4 * t_len * s_len], bf16, tag="qe")
            qev = qe[:].rearrange("p (g t s) -> p g t s", g=g4, t=t_len)
            # (s, t)-ordered view for the packed PSUM->SBUF copies
            qev_st = qe[:].rearrange("p (g t s) -> p g s t", g=g4, t=t_len)

            # ---- load + upcast + gather + regroup ----
            for bt in range(g4):
                g_tile = g_pool.tile([128, 128 * s_len], f32, tag="gt")
                for jj in range(jpt):
                    j = bt * jpt + jj
                    l_tile = load_pool.tile([128, gb * c_dim], bf16, tag="ld")
                    src = (yp.ap()[gb * j:gb * (j + 1)]
                           .rearrange("b t c -> t b c"))
                    dst = l_tile[:].rearrange("p (b c) -> p b c", b=gb)
                    nc.sync.dma_start(dst, src)
                    u_tile = up_pool.tile([128, gb * c_dim], f32, tag="up")
                    nc.scalar.activation(u_tile[:], l_tile[:], Act.Copy)
                    nc.gpsimd.ap_gather(
                        out_ap=g_tile[:, jj * gb * s_len:(jj + 1) * gb * s_len],
                        in_ap=u_tile[:],
                        idxs_ap=gidx_sb[:, j * wc:(j + 1) * wc],
                        channels=128,
                        num_elems=gb * c_dim,
                        d=1,
                        num_idxs=gb * s_len,
                    )
                gv = g_tile[:].rearrange("p (j i s) -> p j i s", j=jpt, i=gb)
                for s0 in range(0, s_len, pk4):
                    ns = min(pk4, s_len - s0)
                    pt = psum_pool.tile([128, 128 * pk4], f32, tag="pt")
                    for k in range(ns):
                        nc.tensor.transpose(pt[:, 128 * k:128 * (k + 1)],
                                            gv[:, :, :, s0 + k], ident[:])
                    src = pt[:].rearrange("p (s t) -> p s t", s=pk4)[:, :ns, :]
                    nc.scalar.activation(qev_st[:, bt, s0:s0 + ns, :], src,
                                         Act.Copy, bias=EPS)

            # ---- DP phase (bf16, prob space, shared-scale rescale) ----
            alpha_a = dp_pool.tile([128, g4 * sg], bf16, tag="alpha_a")
            alpha_b = dp_pool.tile([128, g4 * sg], bf16, tag="alpha_b")
            a_tiles = [alpha_a, alpha_b]
            for a in a_tiles:
                nc.vector.memset(a[:], 0.0)
            av = [a[:].rearrange("p (g s) -> p g s", g=g4) for a in a_tiles]

            u_t = dp_pool.tile([128, g4 * s_len], bf16, tag="u_t")
            v_t = dp_pool.tile([128, g4 * s_len], bf16, tag="v_t")
            uv = u_t[:].rearrange("p (g s) -> p g s", g=g4)
            vv = v_t[:].rearrange("p (g s) -> p g s", g=g4)

            rec = dp_pool.tile([128, nsl], f32, tag="rec")
            mx = dp_pool.tile([128, 1], f32, tag="mx")

            # t = 0 init: alpha[s=0,1] = q'[0, s], rest 0
            nc.vector.tensor_copy(av[0][:, :, 2:4], qev[:, :, 0, 0:2])

            cur = 0
            for t in range(1, t_len):
                prev, nxt = av[cur], av[1 - cur]
                nc.vector.tensor_tensor(uv[:, :, :], prev[:, :, 2:2 + s_len],
                                        prev[:, :, 1:1 + s_len], op=Alu.add)
                veng = (nc.gpsimd if (t % 4) < pool_mask_k else nc.vector)
                veng.tensor_tensor(vv[:, :, :], prev[:, :, 0:s_len],
                                   mv[:, :, :], op=Alu.mult)
                nc.vector.tensor_tensor(uv[:, :, :], uv[:, :, :], vv[:, :, :],
                                        op=Alu.add)
                nc.vector.tensor_tensor(nxt[:, :, 2:2 + s_len], uv[:, :, :],
                                        qev[:, :, t, :], op=Alu.mult)
                if t in resc_ts:
                    slot = resc_ts.index(t)
                    nc.vector.tensor_reduce(mx[:], a_tiles[1 - cur][:, :],
                                            axis=Ax.X, op=Alu.max)
                    nc.vector.reciprocal(rec[:, slot:slot + 1], mx[:])
                    nc.vector.tensor_scalar(
                        a_tiles[1 - cur][:, :], a_tiles[1 - cur][:, :],
                        rec[:, slot:slot + 1], None, op0=Alu.mult)
                cur = 1 - cur

            # ---- epilogue ----
            lg = dp_pool.tile([128, nsl], f32, tag="lg")
            nc.scalar.activation(lg[:], rec[:], Act.Ln)
            lsum = dp_pool.tile([128, 1], f32, tag="lsum")
            nc.vector.tensor_reduce(lsum[:], lg[:], axis=Ax.X, op=Alu.add)
            fin = av[cur]
            tail = dp_pool.tile([128, g4], f32, tag="tail")
            nc.vector.tensor_tensor(tail[:], fin[:, :, sg - 2],
                                    fin[:, :, sg - 1], op=Alu.add)
            ltail = dp_pool.tile([128, g4], f32, tag="ltail")
            nc.scalar.activation(ltail[:], tail[:], Act.Ln)
            # ll = sum_t ln(scale_t) + ln(tail) = -lsum + ltail
            # (lsum = sum ln(rec), rec = 1/scale), so loss = lsum - ltail.
            loss_sb = dp_pool.tile([128, g4], f32, tag="loss_sb")
            nc.vector.tensor_scalar(loss_sb[:], ltail[:], -1.0, lsum[:, 0:1],
                                    op0=Alu.mult, op1=Alu.add)
            nc.sync.dma_start(
                loss.ap().rearrange("(g p) one -> p (g one)", p=128),
                loss_sb[:])

        for _rep in range(repeat):
            body()

    nc.compile()
    return nc


def _host_prep(y_true, y_pred, bc=BC, gb=GB, s_len=S):
    """Shard + build index/mask tensors. Returns in_maps list."""
    import ml_dtypes

    y_true = np.asarray(y_true).astype(np.int64)
    y_pred = np.ascontiguousarray(np.asarray(y_pred), dtype=np.float32)
    yp16 = y_pred.astype(ml_dtypes.bfloat16)
    ncores = y_pred.shape[0] // bc
    g4 = bc // 128
    nbg = bc // gb
    wc = gb * s_len // 16
    ext = np.full((y_true.shape[0], s_len), BLANK, dtype=np.int64)
    ext[:, 1::2] = y_true
    mask_full = np.zeros((ext.shape[0], s_len), dtype=np.float32)
    mask_full[:, 2:] = ((ext[:, 2:] != ext[:, :-2])
                        & (ext[:, 2:] != BLANK)).astype(np.float32)

    in_maps = []
    for cid in range(ncores):
        b0 = cid * bc
        yp_c = yp16[b0:b0 + bc]
        ext_c = ext[b0:b0 + bc]
        # gather indices: per gb-batch chunk j, idxlist[(i, s)] = i*C + ext;
        # wrapped in 16 partitions: stored[p%16, w] = idxlist[w*16 + p%16].
        gidx_c = np.zeros((128, nbg * wc), dtype=np.int16)
        for j in range(nbg):
            idxlist = (np.arange(gb, dtype=np.int16)[:, None] * C
                       + ext_c[gb * j:gb * (j + 1)].astype(np.int16))
            wrapped = idxlist.reshape(-1).reshape(wc, 16).T  # [p16, w]
            gidx_c[:, j * wc:(j + 1) * wc] = np.tile(wrapped, (8, 1))
        m = mask_full[b0:b0 + bc].reshape(g4, 128, s_len).transpose(1, 0, 2)
        mask_c = np.ascontiguousarray(m.reshape(128, g4 * s_len)).astype(
            ml_dtypes.bfloat16)
        in_maps.append({"yp": yp_c, "gidx": gidx_c, "mask": mask_c})
    return in_maps


def get_program(repeat=1):
    key = ("nc", repeat)
    if key not in _CACHE:
        _CACHE[key] = _build_program(repeat=repeat)
    return _CACHE[key]


def kernel(y_true, y_pred):
    from concourse import bass_utils
    nc = get_program()
    in_maps = _host_prep(y_true, y_pred)
    res = bass_utils.run_bass_kernel_spmd(nc, in_maps,
                                          core_ids=list(range(NCORES)))
    out = np.concatenate([res.results[c]["loss"] for c in range(NCORES)],
                         axis=0)
    return out.astype(np.float32)
